# revision 12
# baseline (speedup 1.0000x reference)
"""Trainium2 Bass kernel for 3x3 conv (stride 1, pad 1) + bias.

Problem: x (32,128,56,56) f32, weights (256,128,3,3) f32, bias (256,) f32
         -> out (32,256,56,56) f32.

Strategy: data-parallel over batch (4 images per core, 8 cores), 1-D
Winograd F(2,3) along the width axis.  The width input transform
V[j][row, tx] = sum_b BT[j,b] x[row, 2tx+b] (j = 0..3, tx = 0..27) is
computed on the host and shipped as bf16 (2x the input bytes); the
transformed weights U[kh][j][ci,co] = sum_kw G[j,kw] w[co,ci,kh,kw] are
also host-side.  On device, each of 4 PSUM regions T[j] accumulates the
3 kh taps as plain shifted matmuls over rows (contraction ci=128 on the
partition axis), i.e. the kh convolution stays direct while the kw
convolution runs in the Winograd domain.  That is 12 matmuls of N=392
per (image, 14-row chunk, cout-half) unit -- 6 PE cycles per output
element vs direct conv's 9.  The width output transform
(y_even = T0+T1+T2+bias, y_odd = T1-T2-T3+bias) runs on the Scalar +
Vector + GpSimd engines, writing bf16 with a stride-2 access pattern
that interleaves even/odd columns back into NCHW rows.  Output is DMA'd
bf16 and upcast to fp32 on the host.

Units alternate between two 4-bank PSUM sets, so a unit's column
transform has a full unit's worth of slack to drain before its banks
are reused -- the PE never waits.
"""

import os
from contextlib import ExitStack

import ml_dtypes
import numpy as np

import concourse.bacc as bacc
import concourse.bass as bass
import concourse.mybir as mybir
import concourse.tile as tile
import concourse.bass_utils as bass_utils

N_CORES = 8
B, CIN, H, W = 32, 128, 56, 56
COUT = 256
BPC = B // N_CORES          # images per core
TX = W // 2                 # 28 col-pairs
NJ = 4                      # Winograd width components
PR = H + 2                  # 58 padded rows
RPC = 14                    # output rows per chunk
NCH = H // RPC              # 4 chunks per image
NT = RPC * TX               # 392 (matmul N)
ROWB = NJ * TX              # 112 V elements per row
NSLOT = 24                  # 2 halves x 4 regions x 3 taps

DT = mybir.dt.bfloat16
NPDT = ml_dtypes.bfloat16

# region fill order within a unit: T1 first (feeds the Scalar eviction),
# then T2, T0, T3 -- matches the column-transform consumption order
ORDER1D = (1, 2, 0, 3)

_CACHE: dict = {}


def _build():
    """Build the per-core Bass program (same program on all 8 cores)."""
    nc = bacc.Bacc("TRN2", target_bir_lowering=False, debug=False,
                   num_devices=N_CORES)
    f32 = mybir.dt.float32
    vp = nc.dram_tensor("vp", [BPC, CIN, PR * ROWB], DT,
                        kind="ExternalInput").ap()
    wt = nc.dram_tensor("wt", [CIN, NSLOT * 128], DT,
                        kind="ExternalInput").ap()
    b2 = nc.dram_tensor("b2", [2, 128, 1], f32, kind="ExternalInput").ap()
    out = nc.dram_tensor("out", [BPC, COUT, H, W], DT,
                         kind="ExternalOutput").ap()

    add = mybir.AluOpType.add
    sub = mybir.AluOpType.subtract
    ident = mybir.ActivationFunctionType.Identity

    with tile.TileContext(nc) as tc, ExitStack() as ctx:
        const_pool = ctx.enter_context(tc.tile_pool(name="const", bufs=1))
        vpool = ctx.enter_context(tc.tile_pool(name="vpool", bufs=1))
        spool = ctx.enter_context(tc.tile_pool(name="spool", bufs=1))
        opool = ctx.enter_context(tc.tile_pool(name="opool", bufs=1))
        psum = ctx.enter_context(
            tc.tile_pool(name="psum", bufs=8, space="PSUM"))

        wbuf = const_pool.tile([CIN, NSLOT * 128], DT)
        bbuf = const_pool.tile([128, 2], f32)
        vbufs = [vpool.tile([CIN, PR * ROWB], DT, name=f"v{n}",
                            tag=f"v{n}", bufs=1)
                 for n in range(BPC)]
        pss = [psum.tile([128, NT], f32, name=f"T{i}", tag=f"T{i}", bufs=1)
               for i in range(8)]

        def s_set(k):
            return {nm: spool.tile([128, NT], DT, name=f"{nm}{k}",
                                   tag=f"{nm}{k}", bufs=1)
                    for nm in ("a", "cc", "u", "v")}
        ssets = [s_set(0), s_set(1)]
        obufs = [opool.tile([128, RPC * W], DT, name=f"ob{i}",
                            tag=f"ob{i}", bufs=1)
                 for i in range(3)]

        # HAM warmup fodder -- memset on the vector queue, which is
        # otherwise idle until the first unit's column transform.
        wrm = const_pool.tile([128, NT], DT)
        nc.vector.memset(wrm[:], 0)

        # DMA-in: pieces in exact consumption order, round-robined over
        # TWO in-queues (gpsimd + scalar; sync stays dedicated to
        # output).  Each queue completes in order, so the interleave
        # keeps arrival order ~= need order while doubling the
        # effective early-transfer rate (~160 GB/s per queue).
        pieces = [(wbuf[:, :12 * 128], wt[:, :12 * 128]),
                  (wbuf[:, 12 * 128:], wt[:, 12 * 128:])]
        for n in range(BPC):
            for lo, hi in ((0, 16), (16, 32), (32, 48), (48, PR)):
                pieces.append((vbufs[n][:, lo * ROWB:hi * ROWB],
                               vp[n][:, lo * ROWB:hi * ROWB]))
        for k, (dst, src) in enumerate(pieces):
            (nc.gpsimd if k % 2 == 0 else nc.scalar).dma_start(dst, src)
        for h in range(2):
            nc.scalar.dma_start(bbuf[:, h:h + 1], b2[h])
        VDMA_AT = {}

        # HAM warmup: ~12 junk matmuls while the input DMAs fly.  Uses
        # pss[7], which unit 1 touches last.
        for k in range(12):
            nc.tensor.matmul(pss[7][:], wrm[:, :128], wrm[:],
                             start=(k == 0), stop=(k == 11))

        def stage2_half(ss, ob, base, bias, rlo, rhi):
            """Column transform + bias + bf16 interleave for row range."""
            lo, hi = rlo * TX, rhi * TX
            r = rhi - rlo

            def v3(t):
                return t[:, lo:hi].rearrange("c (r t) -> c r t", t=TX)

            ob3 = ob[:, rlo * W:rhi * W].rearrange(
                "c (r t q) -> c r t q", r=r, t=TX, q=2)
            nc.vector.tensor_tensor(ss["u"][:, lo:hi],
                                    pss[base + 0][:, lo:hi],
                                    ss["a"][:, lo:hi], add)
            nc.vector.tensor_tensor(ss["v"][:, lo:hi],
                                    ss["a"][:, lo:hi],
                                    ss["cc"][:, lo:hi], sub)
            nc.gpsimd.tensor_tensor(ob3[:, :, :, 0], v3(ss["u"]),
                                    v3(ss["cc"]), add)
            nc.vector.tensor_tensor(ob3[:, :, :, 1], v3(ss["v"]),
                                    v3(pss[base + 3]), sub)

        uidx = 0
        for n in range(BPC):
            for ch in range(NCH):
                r0 = ch * RPC
                for h in range(2):
                    base = (uidx % 2) * 4
                    vb3 = vbufs[n][:].rearrange(
                        "c (r j t) -> c r j t", j=NJ, t=TX)
                    # --- GEMM: 4 T-regions x 3 kh taps ---
                    for pos, j in enumerate(ORDER1D):
                        ps = pss[base + j]
                        for kh in range(3):
                            s = h * 12 + pos * 3 + kh
                            nc.tensor.matmul(
                                ps[:],
                                wbuf[:, s * 128:(s + 1) * 128],
                                vb3[:, r0 + kh:r0 + kh + RPC, j, :],
                                start=(kh == 0),
                                stop=(kh == 2),
                            )

                    ss = ssets[uidx % 2]
                    ob = obufs[uidx % 3]
                    if uidx in VDMA_AT:
                        nv = VDMA_AT[uidx]
                        nc.gpsimd.dma_start(vbufs[nv][:], vp[nv])
                    uidx += 1
                    bias = bbuf[:, h:h + 1]
                    nc.scalar.activation(ss["a"][:], pss[base + 1][:],
                                         ident, bias=bias)
                    nc.scalar.activation(ss["cc"][:], pss[base + 2][:],
                                         ident)
                    od = out[n, h * 128:(h + 1) * 128,
                             r0:r0 + RPC, :].rearrange("c r w -> c (r w)")
                    last = (uidx == BPC * NCH * 2)
                    if not last:
                        stage2_half(ss, ob, base, bias, 0, RPC)
                        nc.sync.dma_start(od, ob[:])
                    else:
                        hf = RPC // 2
                        stage2_half(ss, ob, base, bias, 0, hf)
                        nc.sync.dma_start(
                            out[n, h * 128:(h + 1) * 128,
                                r0:r0 + hf, :].rearrange("c r w -> c (r w)"),
                            ob[:, :hf * W])
                        stage2_half(ss, ob, base, bias, hf, RPC)
                        nc.sync.dma_start(
                            out[n, h * 128:(h + 1) * 128,
                                r0 + hf:r0 + RPC, :].rearrange(
                                    "c r w -> c (r w)"),
                            ob[:, hf * W:])
    nc.compile()
    return nc


_BT = np.array([[1, 0, -1, 0], [0, 1, 1, 0],
                [0, -1, 1, 0], [0, 1, 0, -1]], np.float32)
_G = np.array([[1, 0, 0], [.5, .5, .5], [.5, -.5, .5], [0, 0, 1]],
              np.float32)


def _prep(x, weights, bias):
    """Host-side 1-D Winograd width transforms into device layouts."""
    x = np.ascontiguousarray(np.asarray(x, dtype=np.float32))
    xp = np.zeros((B, CIN, PR, W + 2), np.float32)
    xp[:, :, 1:1 + H, 1:1 + W] = x
    d4 = np.lib.stride_tricks.sliding_window_view(
        xp, 4, axis=3)[:, :, :, ::2]                    # [B,C,PR,TX,4]
    V1 = np.einsum('jb,ncrtb->ncrjt', _BT, d4,
                   optimize=True)                       # [B,C,PR,NJ,TX]
    vph = np.ascontiguousarray(V1).astype(NPDT).reshape(
        N_CORES, BPC, CIN, PR * ROWB)

    U1 = np.einsum('jk,ocak->ajco', _G,
                   np.asarray(weights, dtype=np.float32),
                   optimize=True)                       # [kh,j,ci,co]
    wth = np.empty((CIN, NSLOT * 128), np.float32)
    for h in range(2):
        for pos, j in enumerate(ORDER1D):
            for kh in range(3):
                s = h * 12 + pos * 3 + kh
                wth[:, s * 128:(s + 1) * 128] = \
                    U1[kh, j, :, h * 128:(h + 1) * 128]
    wth = wth.astype(NPDT)
    b2 = np.asarray(bias).astype(np.float32).reshape(2, 128, 1)
    return vph, wth, b2


def kernel(x, weights, bias):
    if "nc" not in _CACHE:
        _CACHE["nc"] = _build()
    nc = _CACHE["nc"]
    vph, wth, b2 = _prep(x, weights, bias)
    in_maps = [
        {"vp": vph[i], "wt": wth, "b2": b2}
        for i in range(N_CORES)
    ]
    res = bass_utils.run_bass_kernel_spmd(
        nc, in_maps, core_ids=list(range(N_CORES)),
        trace=bool(int(os.environ.get("CONV_TRACE", "0"))),
    )
    if os.environ.get("CONV_TRACE"):
        _CACHE["last_result"] = res
    return np.concatenate(
        [r["out"] for r in res.results], axis=0).astype(np.float32)
